# revision 7
# baseline (speedup 1.0000x reference)
"""4-bit grouped-quant linear (BitBLAS-style) on 8 TRN2 NeuronCores.

y[m,n] = sum_k x[m,k] * (q[n,k] - zeros[n,g(k)]) * scales[n,g(k)] + bias[n]

Sharding: column-parallel (shard out_features N across 8 cores, replicate x).

Per core (N_shard = 1376), everything in [k, n] layout (host pre-transposes
and unpacks the 4-bit fields into one byte per weight — pure relayout; all
arithmetic on values stays on-device):
  - qTr8[j, t, r, n] holds the uint8 quant value for k = 512t + 4j + r, so
    dequant is a single DVE mult per plane: W'_r = q_r * s (fp16), with the
    scales table host-replicated across the four 32-partition group bands of
    each tile (pure gather/layout).
  - A short dep-less warm-up matmul burst opens the PE HAM clock gate while
    the first input DMAs land.
  - zero-points and bias fold into a rank-33 correction matmul:
        y = x @ (q*s)^T - sum_g zs[n,g] * t_g[m] + bias[n]
    with t_g[m] = sum_{k in g} x[m,k] computed on-device by indicator matmuls
    that double as PE queue filler while dequant runs.
  - Main matmuls: lhsT = x^T plane tiles (stationary), rhs = W' tiles,
    PSUM-accumulated over 32 (t, r) k-tiles + the rank-33 correction; the
    m=0 tile's matmuls are interleaved with dequant production.
  - The last m-tile runs chunk-outer so each N-chunk's correction + PSUM
    drain + store DMA overlaps the next chunk's matmul stream (short tail).
"""

import numpy as np
from contextlib import ExitStack

M, K, N, G = 512, 4096, 11008, 128
NCORES = 8
NS = N // NCORES          # 1376 out-features per core
NT = 8                    # k-tile groups of 512 (each: 4 planes of 128)
R = 4                     # planes per tile group
MT = M // 128             # 4 m-tiles
NCHUNKS = [(0, 512), (512, 512), (1024, 352)]
NWARM = 10                # dep-less warm-up matmuls (N=256 each)


def build_bass():
    import concourse.mybir as mybir
    import concourse.tile as tile
    from concourse import bacc

    f16 = mybir.dt.float16
    f32 = mybir.dt.float32
    u8 = mybir.dt.uint8
    Alu = mybir.AluOpType

    nc = bacc.Bacc(None, target_bir_lowering=False)

    qTr8 = nc.declare_dram_parameter("qTr8", [128, NT, R, NS], u8, isOutput=False)
    xtr4 = nc.declare_dram_parameter("xtr4", [128, NT, R, M], f16, isOutput=False)
    srepr = nc.declare_dram_parameter("srepr", [128, NT, NS], f16, isOutput=False)
    E8 = nc.declare_dram_parameter("E8", [128, NT, 32], f16, isOutput=False)
    sT32 = nc.declare_dram_parameter("sT32", [32, NS], f32, isOutput=False)
    zT32 = nc.declare_dram_parameter("zT32", [32, NS], f32, isOutput=False)
    biasr = nc.declare_dram_parameter("biasr", [1, NS], f32, isOutput=False)
    y = nc.declare_dram_parameter("y", [M, NS], f32, isOutput=True)

    with tile.TileContext(nc) as tc, ExitStack() as ctx:
        persist = ctx.enter_context(tc.tile_pool(name="persist", bufs=1))
        qpool = ctx.enter_context(tc.tile_pool(name="qpool", bufs=2))
        ypool = ctx.enter_context(tc.tile_pool(name="ypool", bufs=2))
        pspool = ctx.enter_context(tc.tile_pool(name="pspool", bufs=6, space="PSUM"))
        tpspool = ctx.enter_context(tc.tile_pool(name="tpspool", bufs=1, space="PSUM"))
        wupool = ctx.enter_context(tc.tile_pool(name="wupool", bufs=1, space="PSUM"))

        # ---- PE warm-up: dep-less matmuls on scratch keep the HAM clock
        # gate open while input DMAs land (PE is idle here anyway) ---------
        wu_sb = persist.tile([128, 256], f16)
        nc.vector.memset(wu_sb, 0.0)
        wu_ps = wupool.tile([128, 256], f32)
        for i in range(NWARM):
            nc.tensor.matmul(wu_ps, wu_sb[:, :128], wu_sb,
                             start=True, stop=True, skip_group_check=True)

        # ---- W' production + x loads + indicator + mi=0 main matmuls -----
        e_sb = persist.tile([128, NT, 32], f16)
        st32_sb = persist.tile([32, NS], f32)
        zt32_sb = persist.tile([32, NS], f32)
        bias_sb = persist.tile([1, NS], f32)
        w4 = persist.tile([128, NT, R, NS], f16)
        x_sb = persist.tile([128, NT, R, M], f16)
        tps = tpspool.tile([32, M], f32)
        ms0 = slice(0, 128)
        pss0 = [pspool.tile([128, 512], f32, tag="ps", name=f"ps_0_{i}")
                for i in range(len(NCHUNKS))]
        nmm_t = 0
        for t in range(NT):
            if t == 0:
                nc.scalar.dma_start(out=e_sb, in_=E8[:, :, :])
            qt8 = qpool.tile([128, R, NS], u8, tag="qt", name=f"qt{t}")
            sr = qpool.tile([128, NS], f16, tag="sr", name=f"sr{t}")
            nc.scalar.dma_start(out=sr, in_=srepr[:, t, :])
            nc.scalar.dma_start(out=x_sb[:, t, :, :], in_=xtr4[:, t, :, :])
            if t == 1:
                nc.scalar.dma_start(out=st32_sb, in_=sT32[:, :])
                nc.scalar.dma_start(out=zt32_sb, in_=zT32[:, :])
                nc.scalar.dma_start(out=bias_sb, in_=biasr[:, :])
            for r in range(R):
                nc.sync.dma_start(out=qt8[:, r, :], in_=qTr8[:, t, r, :])
                # dequant: one DVE mult per plane (host pre-unpacked nibbles)
                nc.vector.tensor_tensor(out=w4[:, t, r, :], in0=qt8[:, r, :],
                                        in1=sr, op=Alu.mult)
                # indicator matmul first: only needs x, fills the PE queue
                # while the dequant mult of this plane completes
                nc.tensor.matmul(tps, e_sb[:, t, :], x_sb[:, t, r, :],
                                 start=(nmm_t == 0), stop=(nmm_t == NT * R - 1))
                for nci, (n0, nsz) in enumerate(NCHUNKS):
                    nc.tensor.matmul(pss0[nci][:, :nsz], x_sb[:, t, r, ms0],
                                     w4[:, t, r, n0:n0 + nsz],
                                     start=(nmm_t == 0), stop=False)
                nmm_t += 1

        # zs33: rows 0..31 = zeros*scales (fp32 mult -> fp16), row 32 = -bias
        zs33 = persist.tile([33, NS], f16)
        nc.vector.tensor_tensor(out=zs33[0:32, :], in0=zt32_sb, in1=st32_sb,
                                op=Alu.mult)
        nc.vector.tensor_scalar(zs33[32:33, :], bias_sb, -1.0, None, Alu.mult)

        # tT33: rows 0..31 = -t_g[m], row 32 = -1
        tT33 = persist.tile([33, M], f16)
        nc.scalar.copy(tT33[0:32, :], tps)
        nc.gpsimd.memset(tT33[32:33, :], -1.0)

        def finish_chunk(mi, nci, n0, nsz, pss):
            ms = slice(mi * 128, (mi + 1) * 128)
            nc.tensor.matmul(pss[:, :nsz], tT33[:, ms],
                             zs33[:, n0:n0 + nsz], start=False, stop=True)
            yt = ypool.tile([128, 512], f32, tag=f"y{nci}", name=f"y{mi}_{nci}")
            eng = (nc.scalar, nc.vector)[nci % 2]
            if eng is nc.scalar:
                eng.copy(yt[:, :nsz], pss[:, :nsz])
            else:
                eng.tensor_copy(yt[:, :nsz], pss[:, :nsz])
            dma = nc.sync if nci % 2 == 0 else nc.scalar
            dma.dma_start(out=y[ms, n0:n0 + nsz], in_=yt[:, :nsz])

        # ---- finish mi=0, then main matmuls for mi=1..3 -------------------
        for nci, (n0, nsz) in enumerate(NCHUNKS):
            finish_chunk(0, nci, n0, nsz, pss0[nci])

        for mi in range(1, MT - 1):
            ms = slice(mi * 128, (mi + 1) * 128)
            pss = [pspool.tile([128, 512], f32, tag="ps", name=f"ps_{mi}_{i}")
                   for i in range(len(NCHUNKS))]
            first = True
            for t in range(NT):
                for r in range(R):
                    for nci, (n0, nsz) in enumerate(NCHUNKS):
                        nc.tensor.matmul(pss[nci][:, :nsz], x_sb[:, t, r, ms],
                                         w4[:, t, r, n0:n0 + nsz],
                                         start=first, stop=False)
                    first = False
            for nci, (n0, nsz) in enumerate(NCHUNKS):
                finish_chunk(mi, nci, n0, nsz, pss[nci])

        # last m-tile: chunk-outer so each chunk's correction + drain + store
        # overlaps the next chunk's matmul stream
        mi = MT - 1
        ms = slice(mi * 128, (mi + 1) * 128)
        for nci, (n0, nsz) in enumerate(NCHUNKS):
            pst = pspool.tile([128, 512], f32, tag="ps", name=f"ps_{mi}_{nci}")
            first = True
            for t in range(NT):
                for r in range(R):
                    nc.tensor.matmul(pst[:, :nsz], x_sb[:, t, r, ms],
                                     w4[:, t, r, n0:n0 + nsz],
                                     start=first, stop=False)
                    first = False
            finish_chunk(mi, nci, n0, nsz, pst)

    nc.finalize()
    return nc


def prep_in_maps(x, qweight, scales, zeros, bias):
    # x planes: xtr4[j, t, r, m] = x[m, 512t + 4j + r]
    xk = x.T.astype(np.float16)                      # [K, M]
    xtr4 = np.ascontiguousarray(
        xk.reshape(NT, 128, R, M).transpose(1, 0, 2, 3))

    E8 = np.zeros((128, NT, 32), np.float16)
    for t in range(NT):
        for j in range(128):
            E8[j, t, 4 * t + j // 32] = -1.0
    # srepr[j, t, n] = scalesT[4t + j//32, n]
    gi = (4 * np.arange(NT)[None, :] + np.arange(128)[:, None] // 32)  # [128, NT]

    in_maps = []
    for c in range(NCORES):
        rows = slice(c * NS, (c + 1) * NS)
        # qTr8[j, t, r, n] = quant nibble for k = 512t + 4j + r  (pure unpack)
        qu8 = qweight[rows].astype(np.uint8).T       # [KP = K//2, NS]
        nib = np.empty((K, qu8.shape[1]), np.uint8)  # nib[k] = 4-bit value
        nib[0::2] = qu8 & 0xF
        nib[1::2] = qu8 >> 4
        qTr8 = np.ascontiguousarray(
            nib.reshape(NT, 128, R, NS).transpose(1, 0, 2, 3))
        sT = np.ascontiguousarray(scales[rows].T)    # [32, NS]
        in_maps.append({
            "qTr8": qTr8,
            "xtr4": xtr4,
            "srepr": np.ascontiguousarray(sT.astype(np.float16)[gi]),
            "E8": E8,
            "sT32": sT.astype(np.float32),
            "zT32": np.ascontiguousarray(zeros[rows].T).astype(np.float32),
            "biasr": bias[rows][None, :].astype(np.float32),
        })
    return in_maps


def kernel(x, qweight, scales, zeros, bias):
    from concourse.bass_utils import run_bass_kernel_spmd

    x = np.asarray(x, dtype=np.float32)
    qweight = np.asarray(qweight)
    scales = np.asarray(scales, dtype=np.float32)
    zeros = np.asarray(zeros, dtype=np.float32)
    bias = np.asarray(bias, dtype=np.float32)

    nc = build_bass()
    in_maps = prep_in_maps(x, qweight, scales, zeros, bias)
    res = run_bass_kernel_spmd(nc, in_maps, list(range(NCORES)))
    return np.concatenate([r["y"] for r in res.results], axis=1)
